# revision 19
# baseline (speedup 1.0000x reference)
"""Multi-head causal attention (B=2, N=2048, D=1024, H=16) on 8 Trainium2 cores.

Sharding: tensor-parallel over heads (2 heads/core) for QKV projections and
attention; per-section AllToAlls redistribute attention outputs to a
token-sharded layout; each core then runs the full output projection for its
512 tokens, pipelined one section behind the attention compute.

Layout: attention computed transposed ("head-dim major"):
  qT/kT/vT = [head_dim(2 heads stacked), tokens]  (from W @ x^T matmuls)
  scores^T = K Q^T per (batch, head)              (the two heads' 64-contract
                                                   matmuls issued adjacently at
                                                   tile_position row halves)
  P^T = exp(scores^T)                             (causal-masked via 0/1 block
                                                   pattern multiplies on DVE)
  attn^T  = (V1^T P)                              (V1 has a ones column -> softmax
                                                   denominator rides along free)
All matmul operands are bf16 (fp32 PSUM accumulation); V is laid out token-major
via DMA xbar transposes instead of PE transposes.

The work is split into 4 sections (batch, q-half of 1024). Each section's QKV
projection chunks are emitted just before its attention, sharing PSUM; its
attention output is normalized, staged, and exchanged with a per-section
AllToAll that overlaps the next section's compute; its output projection is
emitted one section later so the PE never stalls on the collective.
"""

import os

import numpy as np
import ml_dtypes

from concourse import bacc, tile, mybir
import concourse.bass as bass
from concourse.bass_utils import run_bass_kernel_spmd

PAIR = os.environ.get("K_PAIR", "1") == "1"       # adjacent 2-head score MMs
DMATR = os.environ.get("K_DMATR", "1") == "1"     # V via DMA xbar transpose

NCORES = 8
B, N, D, H, HD = 2, 2048, 1024, 16, 64
TOK = B * N              # 4096
HPC = H // NCORES        # 2 heads per core
TPC = TOK // NCORES      # 512 output tokens per core
BT = 128                 # attention block size
NB = N // BT             # 16 key blocks per batch
SW = 1024                # section q width
NSEC = TOK // SW         # 4 sections = (batch, q-half)
CH = 512                 # token chunk for QKV projection matmuls
F32 = mybir.dt.float32
BF16 = mybir.dt.bfloat16
EXP = mybir.ActivationFunctionType.Exp
BF = ml_dtypes.bfloat16


def make_plan(mask):
    """Analyze the [1,1,N,N] mask into per-section/key-block structure."""
    m = np.asarray(mask).reshape(N, N)
    runs = {}
    partial = {}
    patterns = []
    pat_keys = {}
    for kb in range(NB):
        valid_qbs = []
        for qb in range(NB):
            blk = m[qb * BT:(qb + 1) * BT, kb * BT:(kb + 1) * BT]
            if not blk.any():
                continue
            valid_qbs.append(qb)
            if not blk.all():
                pat = np.ascontiguousarray(blk.T.astype(np.float32))
                key = pat.tobytes()
                if key not in pat_keys:
                    pat_keys[key] = len(patterns)
                    patterns.append(pat)
                partial.setdefault(kb, []).append((qb, pat_keys[key]))
        rr = []
        for qb in valid_qbs:
            if rr and rr[-1][1] == qb * BT:
                rr[-1][1] = (qb + 1) * BT
            else:
                rr.append([qb * BT, (qb + 1) * BT])
        runs[kb] = [tuple(r) for r in rr]
    if not patterns:
        patterns.append(np.ones((BT, BT), np.float32))

    # per-section tables (section s = (batch s//2, q-half s%2), local q coords)
    sec_pieces = []   # [s][kb] -> list of (c0, c1) local, split at 512
    sec_partial = []  # [s] -> list of (qc_local, pidx)
    sec_fl = []       # [s][cc] -> (first_kb, last_kb)
    for s in range(NSEC):
        q0 = (s % 2) * SW
        pieces_by_kb = {}
        fl = [[None, None] for _ in range(SW // 512)]
        for kb in range(NB):
            pieces = []
            for (r0, r1) in runs.get(kb, []):
                lo, hi = max(r0, q0), min(r1, q0 + SW)
                c = lo
                while c < hi:
                    e = min(((c - q0) // 512 + 1) * 512 + q0, hi)
                    pieces.append((c - q0, e - q0))
                    c = e
            if pieces:
                pieces_by_kb[kb] = pieces
                for (c0, c1) in pieces:
                    cc = c0 // 512
                    if fl[cc][0] is None:
                        fl[cc][0] = kb
                    fl[cc][1] = kb
        sec_pieces.append(pieces_by_kb)
        parts = []
        for kb, lst in partial.items():
            for (qb, pidx) in lst:
                qc = qb * BT
                if q0 <= qc < q0 + SW:
                    parts.append((kb, qc - q0, pidx))
        sec_partial.append(parts)
        sec_fl.append(fl)
    return {
        "patterns": np.stack(patterns).astype(BF),
        "sec_pieces": sec_pieces,
        "sec_partial": sec_partial,
        "sec_fl": sec_fl,
    }


def build_nc(plan):
    nc = bacc.Bacc("TRN2", target_bir_lowering=False, debug=False,
                   num_devices=NCORES)
    n_pat = plan["patterns"].shape[0]

    xT = nc.dram_tensor("xT", [D, TOK], BF16, kind="ExternalInput")
    wqT = nc.dram_tensor("wqT", [D, BT], BF16, kind="ExternalInput")
    wkT = nc.dram_tensor("wkT", [D, BT], BF16, kind="ExternalInput")
    wvT = nc.dram_tensor("wvT", [D, BT], BF16, kind="ExternalInput")
    woT = nc.dram_tensor("woT", [D, D], BF16, kind="ExternalInput")
    bo1 = nc.dram_tensor("bo1", [1, D], F32, kind="ExternalInput")
    pm = nc.dram_tensor("pm", [n_pat, BT, BT], BF16, kind="ExternalInput")
    ident = nc.dram_tensor("ident", [BT, BT], F32, kind="ExternalInput")
    y = nc.dram_tensor("y", [TPC, D], F32, kind="ExternalOutput")

    sec_pieces, sec_partial = plan["sec_pieces"], plan["sec_partial"]
    sec_fl = plan["sec_fl"]

    with tile.TileContext(nc) as tc:
        with (
            tc.tile_pool(name="const", bufs=1) as cp,
            tc.tile_pool(name="big", bufs=1) as bigp,
            tc.tile_pool(name="psum", bufs=1, space="PSUM") as psum,
            tc.tile_pool(name="dram", bufs=1, space="DRAM") as dram,
        ):
            # ---- constants (issued on scalar: sync leads with x chunks) ----
            pmt = [cp.tile([BT, BT], BF16, name=f"pmt{i}") for i in range(n_pat)]
            for i in range(n_pat):
                nc.scalar.dma_start(pmt[i][:], pm.ap()[i])
            identt = cp.tile([BT, BT], F32, name="identt")
            nc.scalar.dma_start(identt[:], ident.ap())
            bot = cp.tile([1, D], F32, name="bot")
            nc.scalar.dma_start(bot[:], bo1.ap())
            bobc = cp.tile([BT, D], F32, name="bobc")
            nc.gpsimd.partition_broadcast(bobc[:], bot[:])

            # ---- warm-up collective (absorbs launch skew / setup) ----
            wa_sb = cp.tile([BT, 4], F32, name="wa_sb")
            nc.vector.memset(wa_sb[:], 1.0)
            wa_in = dram.tile([BT, 4], F32, name="wa_in")
            wa_out = dram.tile([BT * NCORES, 4], F32, name="wa_out",
                               addr_space="Shared")
            nc.scalar.dma_start(wa_in[:], wa_sb[:])
            nc.gpsimd.collective_compute(
                "AllGather", mybir.AluOpType.bypass,
                ins=[wa_in.opt()], outs=[wa_out.opt()],
                replica_groups=[list(range(NCORES))])

            a2a_in = [dram.tile([NCORES * BT, BT], BF16, name=f"a2a_in{s}")
                      for s in range(NSEC)]
            a2a_out = [dram.tile([NCORES * BT, BT], BF16, name=f"a2a_out{s}")
                       for s in range(NSEC)]

            qTt = bigp.tile([BT, TOK], BF16, name="qTt")
            kTt = bigp.tile([BT, TOK], BF16, name="kTt")
            # V token-major: per (head, token-block) [128 tok, 64 hd + ones]
            v1h = {}
            for j in range(HPC):
                for tb in range(TOK // BT):
                    t = bigp.tile([BT, HD + 1], BF16, name=f"v1_{j}_{tb}")
                    v1h[(j, tb)] = t
                    nc.gpsimd.memset(t[:, HD:HD + 1], 1.0)

            with (
                tc.tile_pool(name="wqkv", bufs=1) as wp,
                tc.tile_pool(name="xp", bufs=2) as xp,
                tc.tile_pool(name="vtp", bufs=2) as vtp,
                tc.tile_pool(name="ptp", bufs=2) as ptp,
                tc.tile_pool(name="workp", bufs=2) as workp,
            ):
                # fused single-trigger weight loads (row-blocks side by side)
                wq_all = wp.tile([BT, 8 * BT], BF16, name="wq_all")
                wk_all = wp.tile([BT, 8 * BT], BF16, name="wk_all")
                wv_all = wp.tile([BT, 8 * BT], BF16, name="wv_all")
                for (wt, src) in ((wq_all, wqT), (wk_all, wkT), (wv_all, wvT)):
                    nc.scalar.dma_start(
                        wt[:].rearrange("p (g c) -> p g c", g=8),
                        src.ap().rearrange("(g p) c -> p g c", p=BT))
                wq = [wq_all[:, e * BT:(e + 1) * BT] for e in range(8)]
                wk = [wk_all[:, e * BT:(e + 1) * BT] for e in range(8)]
                wv = [wv_all[:, e * BT:(e + 1) * BT] for e in range(8)]
                wo_all = wp.tile([BT, 8 * D], BF16, name="wo_all")
                nc.scalar.dma_start(
                    wo_all[:].rearrange("p (g c) -> p g c", g=8),
                    woT.ap().rearrange("(g p) c -> p g c", p=BT))

                def emit_qkv_chunk(ch):
                    xt_all = xp.tile([BT, 8 * CH], BF16, name="xt", tag="xt")
                    nc.sync.dma_start(
                        xt_all[:].rearrange("p (g c) -> p g c", g=8),
                        xT.ap()[:, ch * CH:(ch + 1) * CH]
                        .rearrange("(g p) c -> p g c", p=BT))
                    xt = [xt_all[:, e * CH:(e + 1) * CH] for e in range(8)]
                    vtmp = vtp.tile([BT, CH], F32, name="vtmp", tag="vtmp")
                    for (wt, dst) in ((wq, qTt), (wk, kTt), (wv, vtmp)):
                        ps = psum.tile([BT, CH], F32, name="psqkv", tag="psqkv",
                                       bufs=2)
                        for e in range(8):
                            nc.tensor.matmul(ps[:], wt[e], xt[e],
                                             start=(e == 0), stop=(e == 7))
                        if dst is vtmp:
                            nc.vector.tensor_copy(vtmp[:], ps[:])
                        else:
                            dslc = dst[:, ch * CH:(ch + 1) * CH]
                            nc.vector.tensor_copy(dslc, ps[:])
                    # V token-major via PE transpose (f32, borrows psS slots)
                    for k in range(CH // BT):
                        tb = ch * (CH // BT) + k
                        pst = psum.tile([BT, 512], F32, name="psS",
                                        tag="psS", bufs=2)
                        nc.tensor.transpose(
                            pst[:, 0:BT], vtmp[:, k * BT:(k + 1) * BT],
                            identt[:])
                        for j in range(HPC):
                            nc.vector.tensor_copy(
                                v1h[(j, tb)][:, 0:HD],
                                pst[:, HD * j:HD * (j + 1)])

                def emit_outproj(s):
                    aT_all = workp.tile([BT, 8 * BT], BF16, name="aT",
                                        tag="aT")
                    nc.sync.dma_start(
                        aT_all[:].rearrange("p (i c) -> p i c", i=8),
                        a2a_out[s][:].rearrange("(i p) c -> p i c", p=BT))
                    aTs = [aT_all[:, i * BT:(i + 1) * BT] for i in range(8)]
                    for oc in range(D // 512):
                        psY = psum.tile([BT, 512], F32, name="psY", tag="psqkv",
                                        bufs=2)
                        for i in range(8):
                            nc.tensor.matmul(
                                psY[:], aTs[i],
                                wo_all[:, i * D + oc * 512:
                                       i * D + (oc + 1) * 512],
                                start=(i == 0), stop=(i == 7))
                        ysb = workp.tile([BT, 512], F32, name="ysb", tag="ysb")
                        nc.vector.tensor_add(ysb[:], psY[:],
                                             bobc[:, oc * 512:(oc + 1) * 512])
                        nc.sync.dma_start(
                            y.ap()[BT * s:BT * (s + 1),
                                   oc * 512:(oc + 1) * 512],
                            ysb[:])

                for s in range(NSEC):
                    b = s // 2
                    q0 = (s % 2) * SW
                    emit_qkv_chunk(2 * s)
                    emit_qkv_chunk(2 * s + 1)

                    psO = [psum.tile([HD + 1, SW], F32, name=f"psO{j}",
                                     tag=f"psO{j}") for j in range(HPC)]
                    pieces_by_kb = sec_pieces[s]
                    fl = sec_fl[s]
                    parts = sec_partial[s]
                    for kb in range(NB):
                        pieces = pieces_by_kb.get(kb)
                        if not pieces:
                            continue
                        kcols = slice(b * N + kb * BT, b * N + (kb + 1) * BT)
                        pT = [ptp.tile([BT, SW], BF16, name=f"pT{j}",
                                       tag=f"pT{j}") for j in range(HPC)]
                        for (c0, c1) in pieces:
                            w = c1 - c0
                            qcols = slice(b * N + q0 + c0, b * N + q0 + c1)
                            if PAIR:
                                psS = []
                                for j in range(HPC):
                                    ps = psum.tile([BT, 512], F32, name="psS",
                                                   tag="psS",
                                                   bufs=2 if DMATR else 1)
                                    psS.append(ps)
                                # the two heads' 64-contract matmuls issued
                                # back-to-back at distinct PE row halves
                                for j in range(HPC):
                                    ks = slice(HD * j, HD * (j + 1))
                                    nc.tensor.matmul(
                                        psS[j][:, 0:w], kTt[ks, kcols],
                                        qTt[ks, qcols], start=True, stop=True,
                                        tile_position=(HD * j, 0))
                                for j in range(HPC):
                                    nc.scalar.activation(
                                        pT[j][:, c0:c1], psS[j][:, 0:w],
                                        EXP, scale=0.125)
                            else:
                                for j in range(HPC):
                                    ks = slice(HD * j, HD * (j + 1))
                                    ps = psum.tile([BT, 512], F32, name="psS",
                                                   tag="psS",
                                                   bufs=2 if DMATR else 1)
                                    nc.tensor.matmul(
                                        ps[:, 0:w], kTt[ks, kcols],
                                        qTt[ks, qcols], start=True, stop=True,
                                        tile_position=(HD * j, 0))
                                    nc.scalar.activation(
                                        pT[j][:, c0:c1], ps[:, 0:w],
                                        EXP, scale=0.125)
                        # partial-block masking (0/1 pattern multiply on DVE)
                        for (pkb, qc, pidx) in parts:
                            if pkb != kb:
                                continue
                            for j in range(HPC):
                                slc = pT[j][:, qc:qc + BT]
                                nc.vector.tensor_mul(slc, slc, pmt[pidx][:])
                        # AV accumulate (+ denominator via ones column)
                        v1t = v1h
                        for j in range(HPC):
                            vt = v1h[(j, b * NB + kb)]
                            for (c0, c1) in pieces:
                                cc = c0 // 512
                                nc.tensor.matmul(
                                    psO[j][:, c0:c1], vt[:],
                                    pT[j][:, c0:c1],
                                    start=(kb == fl[cc][0]),
                                    stop=(kb == fl[cc][1]))
                    # normalize: 1/denominator, broadcast, fused evacuation
                    rcp = [workp.tile([1, SW], F32, name=f"rcp{j}",
                                      tag=f"rcp{j}") for j in range(HPC)]
                    for j in range(HPC):
                        dnj = workp.tile([1, SW], F32, name=f"dn{j}",
                                         tag=f"dn{j}")
                        nc.vector.tensor_copy(dnj[:], psO[j][HD:HD + 1, :])
                        nc.vector.reciprocal_approx_fast(rcp[j][:], dnj[:])
                    rbc = [workp.tile([HD, SW], F32, name=f"rbc{j}",
                                      tag=f"rbc{j}") for j in range(HPC)]
                    for j in range(HPC):
                        nc.gpsimd.partition_broadcast(rbc[j][:], rcp[j][:])
                    attnS = [workp.tile([HD, SW], BF16, name=f"attnS{j}",
                                        tag=f"attnS{j}") for j in range(HPC)]
                    for j in range(HPC):
                        nc.vector.tensor_mul(attnS[j][:], psO[j][0:HD, :],
                                             rbc[j][:])
                    # stage for the per-section AllToAll: dest core r gets
                    # tokens [q0 + 128r, q0 + 128(r+1)) of batch b
                    for j in range(HPC):
                        nc.sync.dma_start(
                            a2a_in[s][:]
                            .rearrange("(r h p) c -> h p r c", r=NCORES,
                                       h=HPC)[j],
                            attnS[j][:].rearrange("p (r c) -> p r c",
                                                  r=NCORES))
                    nc.gpsimd.collective_compute(
                        "AllToAll", mybir.AluOpType.bypass,
                        ins=[a2a_in[s].opt()], outs=[a2a_out[s].opt()],
                        replica_groups=[list(range(NCORES))])
                    if s >= 1:
                        emit_outproj(s - 1)
                emit_outproj(NSEC - 1)
    nc.compile()
    return nc


_CACHE = {}


def _get_nc(plan_key, mask):
    if plan_key not in _CACHE:
        _CACHE[plan_key] = build_nc(make_plan(mask))
    return _CACHE[plan_key]


def _prep_inputs(x, mask, Wq, Wk, Wv, Wo, bo, plan):
    xT = np.ascontiguousarray(x.reshape(TOK, D).T).astype(BF)
    woT = np.ascontiguousarray(Wo.T).astype(BF)
    bo1 = np.ascontiguousarray(bo[None, :]).astype(np.float32)
    in_maps = []
    for c in range(NCORES):
        rows = slice(BT * c, BT * (c + 1))
        in_maps.append({
            "xT": xT,
            "wqT": np.ascontiguousarray(Wq[rows].T).astype(BF),
            "wkT": np.ascontiguousarray(Wk[rows].T).astype(BF),
            "wvT": np.ascontiguousarray(Wv[rows].T).astype(BF),
            "woT": woT,
            "bo1": bo1,
            "pm": plan["patterns"],
            "ident": np.eye(BT, dtype=np.float32),
        })
    return in_maps


def run(inputs, trace=False, **kw):
    x = np.asarray(inputs["x"], np.float32)
    mask = np.asarray(inputs["mask"])
    plan_key = mask.tobytes()
    nc = _get_nc(plan_key, mask)
    plan = make_plan(mask)
    in_maps = _prep_inputs(x, mask, np.asarray(inputs["Wq"], np.float32),
                           np.asarray(inputs["Wk"], np.float32),
                           np.asarray(inputs["Wv"], np.float32),
                           np.asarray(inputs["Wo"], np.float32),
                           np.asarray(inputs["bo"], np.float32), plan)
    res = run_bass_kernel_spmd(nc, in_maps, core_ids=list(range(NCORES)),
                               trace=trace, **kw)
    # unshard: core c, section s, row i -> global token s*1024 + 128c + i
    out = np.empty((TOK, D), np.float32)
    for c in range(NCORES):
        yc = res.results[c]["y"]
        for s in range(NSEC):
            out[s * SW + BT * c: s * SW + BT * (c + 1)] = \
                yc[BT * s: BT * (s + 1)]
    return out.reshape(B, N, D), res


def kernel(**inputs):
    out, _ = run(inputs, trace=False)
    return out


# revision 24
# speedup vs baseline: 1.0546x; 1.0546x over previous
"""Multi-head causal attention (B=2, N=2048, D=1024, H=16) on 8 Trainium2 cores.

Sharding: tensor-parallel over heads (2 heads/core) for QKV projections and
attention; per-section AllToAlls redistribute attention outputs to a
token-sharded layout; each core then runs the full output projection for its
512 tokens, pipelined one section behind the attention compute.

Layout: attention computed transposed ("head-dim major"):
  qT/kT/vT = [head_dim(2 heads stacked), tokens]  (from W @ x^T matmuls)
  scores^T = K Q^T per (batch, head)              (the two heads' 64-contract
                                                   matmuls issued adjacently at
                                                   tile_position row halves)
  P^T = exp(scores^T)                             (causal-masked via 0/1 block
                                                   pattern multiplies on DVE)
  attn^T  = (V1^T P)                              (V1 has a ones column -> softmax
                                                   denominator rides along free)
All matmul operands are bf16 (fp32 PSUM accumulation); V is laid out token-major
via DMA xbar transposes instead of PE transposes.

The work is split into 4 sections (batch, q-half of 1024). Each section's QKV
projection chunks are emitted just before its attention, sharing PSUM; its
attention output is normalized, staged, and exchanged with a per-section
AllToAll that overlaps the next section's compute; its output projection is
emitted one section later so the PE never stalls on the collective.
"""

import os

import numpy as np
import ml_dtypes

from concourse import bacc, tile, mybir
import concourse.bass as bass
from concourse.bass_utils import run_bass_kernel_spmd

PAIR = os.environ.get("K_PAIR", "1") == "1"       # adjacent 2-head score MMs
DMATR = os.environ.get("K_DMATR", "1") == "1"     # V via DMA xbar transpose

NCORES = 8
B, N, D, H, HD = 2, 2048, 1024, 16, 64
TOK = B * N              # 4096
HPC = H // NCORES        # 2 heads per core
TPC = TOK // NCORES      # 512 output tokens per core
BT = 128                 # attention block size
NB = N // BT             # 16 key blocks per batch
SW = 1024                # section q width
NSEC = TOK // SW         # 4 sections = (batch, q-half)
CH = 512                 # token chunk for QKV projection matmuls
F32 = mybir.dt.float32
BF16 = mybir.dt.bfloat16
EXP = mybir.ActivationFunctionType.Exp
BF = ml_dtypes.bfloat16


def make_plan(mask):
    """Analyze the [1,1,N,N] mask into per-section/key-block structure."""
    m = np.asarray(mask).reshape(N, N)
    runs = {}
    partial = {}
    patterns = []
    pat_keys = {}
    for kb in range(NB):
        valid_qbs = []
        for qb in range(NB):
            blk = m[qb * BT:(qb + 1) * BT, kb * BT:(kb + 1) * BT]
            if not blk.any():
                continue
            valid_qbs.append(qb)
            if not blk.all():
                pat = np.ascontiguousarray(blk.T.astype(np.float32))
                key = pat.tobytes()
                if key not in pat_keys:
                    pat_keys[key] = len(patterns)
                    patterns.append(pat)
                partial.setdefault(kb, []).append((qb, pat_keys[key]))
        rr = []
        for qb in valid_qbs:
            if rr and rr[-1][1] == qb * BT:
                rr[-1][1] = (qb + 1) * BT
            else:
                rr.append([qb * BT, (qb + 1) * BT])
        runs[kb] = [tuple(r) for r in rr]
    if not patterns:
        patterns.append(np.ones((BT, BT), np.float32))

    # per-section tables (section s = (batch s//2, q-half s%2), local q coords)
    sec_pieces = []   # [s][kb] -> list of (c0, c1) local, split at 512
    sec_partial = []  # [s] -> list of (qc_local, pidx)
    sec_fl = []       # [s][cc] -> (first_kb, last_kb)
    for s in range(NSEC):
        q0 = (s % 2) * SW
        pieces_by_kb = {}
        fl = [[None, None] for _ in range(SW // 512)]
        for kb in range(NB):
            pieces = []
            for (r0, r1) in runs.get(kb, []):
                lo, hi = max(r0, q0), min(r1, q0 + SW)
                c = lo
                while c < hi:
                    e = min(((c - q0) // 512 + 1) * 512 + q0, hi)
                    pieces.append((c - q0, e - q0))
                    c = e
            if pieces:
                pieces_by_kb[kb] = pieces
                for (c0, c1) in pieces:
                    cc = c0 // 512
                    if fl[cc][0] is None:
                        fl[cc][0] = kb
                    fl[cc][1] = kb
        sec_pieces.append(pieces_by_kb)
        parts = []
        for kb, lst in partial.items():
            for (qb, pidx) in lst:
                qc = qb * BT
                if q0 <= qc < q0 + SW:
                    parts.append((kb, qc - q0, pidx))
        sec_partial.append(parts)
        sec_fl.append(fl)
    return {
        "patterns": np.stack(patterns).astype(BF),
        "sec_pieces": sec_pieces,
        "sec_partial": sec_partial,
        "sec_fl": sec_fl,
    }


def build_nc(plan):
    nc = bacc.Bacc("TRN2", target_bir_lowering=False, debug=False,
                   num_devices=NCORES)
    n_pat = plan["patterns"].shape[0]

    xT = nc.dram_tensor("xT", [D, TOK], BF16, kind="ExternalInput")
    wqT = nc.dram_tensor("wqT", [D, BT], BF16, kind="ExternalInput")
    wkT = nc.dram_tensor("wkT", [D, BT], BF16, kind="ExternalInput")
    wvT = nc.dram_tensor("wvT", [D, BT], BF16, kind="ExternalInput")
    woT = nc.dram_tensor("woT", [D, D], BF16, kind="ExternalInput")
    bo1 = nc.dram_tensor("bo1", [1, D], F32, kind="ExternalInput")
    pm = nc.dram_tensor("pm", [n_pat, BT, BT], BF16, kind="ExternalInput")
    ident = nc.dram_tensor("ident", [BT, BT], F32, kind="ExternalInput")
    y = nc.dram_tensor("y", [TPC, D], F32, kind="ExternalOutput")

    sec_pieces, sec_partial = plan["sec_pieces"], plan["sec_partial"]
    sec_fl = plan["sec_fl"]

    with tile.TileContext(nc) as tc:
        with (
            tc.tile_pool(name="const", bufs=1) as cp,
            tc.tile_pool(name="big", bufs=1) as bigp,
            tc.tile_pool(name="psum", bufs=1, space="PSUM") as psum,
            tc.tile_pool(name="dram", bufs=1, space="DRAM") as dram,
        ):
            # ---- constants (issued on scalar: sync leads with x chunks) ----
            pmt = [cp.tile([BT, BT], BF16, name=f"pmt{i}") for i in range(n_pat)]
            for i in range(n_pat):
                nc.scalar.dma_start(pmt[i][:], pm.ap()[i])
            identt = cp.tile([BT, BT], F32, name="identt")
            nc.scalar.dma_start(identt[:], ident.ap())
            bot = cp.tile([1, D], F32, name="bot")
            nc.scalar.dma_start(bot[:], bo1.ap())
            bobc = cp.tile([BT, D], F32, name="bobc")
            nc.gpsimd.partition_broadcast(bobc[:], bot[:])

            # ---- warm-up collective (absorbs launch skew / setup) ----
            wa_sb = cp.tile([BT, 4], F32, name="wa_sb")
            nc.vector.memset(wa_sb[:], 1.0)
            wa_in = dram.tile([BT, 4], F32, name="wa_in")
            wa_out = dram.tile([BT * NCORES, 4], F32, name="wa_out",
                               addr_space="Shared")
            nc.scalar.dma_start(wa_in[:], wa_sb[:])
            nc.gpsimd.collective_compute(
                "AllGather", mybir.AluOpType.bypass,
                ins=[wa_in.opt()], outs=[wa_out.opt()],
                replica_groups=[list(range(NCORES))])

            # one AllToAll per (section, 512-token half): the first half's
            # exchange fires mid-section, only the last half rides the tail
            a2a_in = {(s, h): dram.tile([NCORES * BT, HD], BF16,
                                        name=f"a2a_in{s}_{h}")
                      for s in range(NSEC) for h in range(2)}
            a2a_out = {(s, h): dram.tile([NCORES * BT, HD], BF16,
                                         name=f"a2a_out{s}_{h}")
                       for s in range(NSEC) for h in range(2)}

            qTt = bigp.tile([BT, TOK], BF16, name="qTt")
            kTt = bigp.tile([BT, TOK], BF16, name="kTt")
            # V token-major: per (head, token-block) [128 tok, 64 hd + ones]
            v1h = {}
            for j in range(HPC):
                for tb in range(TOK // BT):
                    t = bigp.tile([BT, HD + 1], BF16, name=f"v1_{j}_{tb}")
                    v1h[(j, tb)] = t
                    nc.gpsimd.memset(t[:, HD:HD + 1], 1.0)

            with (
                tc.tile_pool(name="wqkv", bufs=1) as wp,
                tc.tile_pool(name="xp", bufs=2) as xp,
                tc.tile_pool(name="vtp", bufs=2) as vtp,
                tc.tile_pool(name="ptp", bufs=2) as ptp,
                tc.tile_pool(name="workp", bufs=2) as workp,
            ):
                # fused single-trigger weight loads (row-blocks side by side)
                wq_all = wp.tile([BT, 8 * BT], BF16, name="wq_all")
                wk_all = wp.tile([BT, 8 * BT], BF16, name="wk_all")
                wv_all = wp.tile([BT, 8 * BT], BF16, name="wv_all")
                for (wt, src) in ((wq_all, wqT), (wk_all, wkT), (wv_all, wvT)):
                    nc.scalar.dma_start(
                        wt[:].rearrange("p (g c) -> p g c", g=8),
                        src.ap().rearrange("(g p) c -> p g c", p=BT))
                wq = [wq_all[:, e * BT:(e + 1) * BT] for e in range(8)]
                wk = [wk_all[:, e * BT:(e + 1) * BT] for e in range(8)]
                wv = [wv_all[:, e * BT:(e + 1) * BT] for e in range(8)]
                wo_all = wp.tile([BT, 8 * D], BF16, name="wo_all")
                nc.scalar.dma_start(
                    wo_all[:].rearrange("p (g c) -> p g c", g=8),
                    woT.ap().rearrange("(g p) c -> p g c", p=BT))

                def emit_qkv_chunk(ch):
                    xt_all = xp.tile([BT, 8 * CH], BF16, name="xt", tag="xt")
                    nc.sync.dma_start(
                        xt_all[:].rearrange("p (g c) -> p g c", g=8),
                        xT.ap()[:, ch * CH:(ch + 1) * CH]
                        .rearrange("(g p) c -> p g c", p=BT))
                    xt = [xt_all[:, e * CH:(e + 1) * CH] for e in range(8)]
                    vtmp = vtp.tile([BT, CH], F32, name="vtmp", tag="vtmp")
                    for (wt, dst) in ((wq, qTt), (wk, kTt), (wv, vtmp)):
                        ps = psum.tile([BT, CH], F32, name="psqkv", tag="psqkv",
                                       bufs=2)
                        for e in range(8):
                            nc.tensor.matmul(ps[:], wt[e], xt[e],
                                             start=(e == 0), stop=(e == 7))
                        if dst is vtmp:
                            nc.vector.tensor_copy(vtmp[:], ps[:])
                        else:
                            dslc = dst[:, ch * CH:(ch + 1) * CH]
                            nc.vector.tensor_copy(dslc, ps[:])
                    # V token-major via PE transpose (f32, borrows psS slots)
                    for k in range(CH // BT):
                        tb = ch * (CH // BT) + k
                        pst = psum.tile([BT, 512], F32, name="psS",
                                        tag="psS", bufs=2)
                        nc.tensor.transpose(
                            pst[:, 0:BT], vtmp[:, k * BT:(k + 1) * BT],
                            identt[:])
                        for j in range(HPC):
                            nc.vector.tensor_copy(
                                v1h[(j, tb)][:, 0:HD],
                                pst[:, HD * j:HD * (j + 1)])

                def emit_outproj(s):
                    aT_all = workp.tile([BT, 8 * BT], BF16, name="aT",
                                        tag="aT")
                    for h in range(2):
                        nc.sync.dma_start(
                            aT_all[:].rearrange("p (i g c) -> g p i c",
                                                i=8, g=2)[h],
                            a2a_out[(s, h)][:]
                            .rearrange("(i p) c -> p i c", p=BT))
                    aTs = [aT_all[:, i * BT:(i + 1) * BT] for i in range(8)]
                    for oc in range(D // 512):
                        psY = psum.tile([BT, 512], F32, name="psY", tag="psqkv",
                                        bufs=2)
                        for i in range(8):
                            nc.tensor.matmul(
                                psY[:], aTs[i],
                                wo_all[:, i * D + oc * 512:
                                       i * D + (oc + 1) * 512],
                                start=(i == 0), stop=(i == 7))
                        ysb = workp.tile([BT, 512], F32, name="ysb", tag="ysb")
                        nc.vector.tensor_add(ysb[:], psY[:],
                                             bobc[:, oc * 512:(oc + 1) * 512])
                        nc.sync.dma_start(
                            y.ap()[BT * s:BT * (s + 1),
                                   oc * 512:(oc + 1) * 512],
                            ysb[:])

                for s in range(NSEC):
                    b = s // 2
                    q0 = (s % 2) * SW
                    emit_qkv_chunk(2 * s)
                    emit_qkv_chunk(2 * s + 1)

                    psO = [psum.tile([HD + 1, SW], F32, name=f"psO{j}",
                                     tag=f"psO{j}") for j in range(HPC)]
                    pieces_by_kb = sec_pieces[s]
                    fl = sec_fl[s]
                    parts = sec_partial[s]

                    def emit_half(h):
                        # normalize + stage + exchange one 512-token half
                        hs = slice(512 * h, 512 * (h + 1))
                        attnS = []
                        for j in range(HPC):
                            dnj = workp.tile([1, 512], F32, name=f"dn{j}",
                                             tag=f"dn{j}")
                            nc.vector.tensor_copy(dnj[:],
                                                  psO[j][HD:HD + 1, hs])
                            rcp = workp.tile([1, 512], F32, name=f"rcp{j}",
                                             tag=f"rcp{j}")
                            nc.vector.reciprocal_approx_fast(rcp[:], dnj[:])
                            rbc = workp.tile([HD, 512], F32, name=f"rbc{j}",
                                             tag=f"rbc{j}")
                            nc.gpsimd.partition_broadcast(rbc[:], rcp[:])
                            at = workp.tile([HD, 512], BF16, name=f"attnS{j}",
                                            tag=f"attnS{j}")
                            nc.vector.tensor_mul(at[:], psO[j][0:HD, hs],
                                                 rbc[:])
                            attnS.append(at)
                        # dest core r gets tokens [512h + 64r, 512h + 64(r+1))
                        for j in range(HPC):
                            nc.sync.dma_start(
                                a2a_in[(s, h)][:]
                                .rearrange("(r g p) c -> g p r c", r=NCORES,
                                           g=HPC)[j],
                                attnS[j][:].rearrange("p (r c) -> p r c",
                                                      r=NCORES))
                        nc.gpsimd.collective_compute(
                            "AllToAll", mybir.AluOpType.bypass,
                            ins=[a2a_in[(s, h)].opt()],
                            outs=[a2a_out[(s, h)].opt()],
                            replica_groups=[list(range(NCORES))])

                    for kb in range(NB):
                        pieces = pieces_by_kb.get(kb)
                        if not pieces:
                            continue
                        kcols = slice(b * N + kb * BT, b * N + (kb + 1) * BT)
                        pT = [ptp.tile([BT, SW], BF16, name=f"pT{j}",
                                       tag=f"pT{j}") for j in range(HPC)]
                        for (c0, c1) in pieces:
                            w = c1 - c0
                            qcols = slice(b * N + q0 + c0, b * N + q0 + c1)
                            if PAIR:
                                psS = []
                                for j in range(HPC):
                                    ps = psum.tile([BT, 512], F32, name="psS",
                                                   tag="psS",
                                                   bufs=2 if DMATR else 1)
                                    psS.append(ps)
                                # the two heads' 64-contract matmuls issued
                                # back-to-back at distinct PE row halves
                                for j in range(HPC):
                                    ks = slice(HD * j, HD * (j + 1))
                                    nc.tensor.matmul(
                                        psS[j][:, 0:w], kTt[ks, kcols],
                                        qTt[ks, qcols], start=True, stop=True,
                                        tile_position=(HD * j, 0))
                                for j in range(HPC):
                                    nc.scalar.activation(
                                        pT[j][:, c0:c1], psS[j][:, 0:w],
                                        EXP, scale=0.125)
                            else:
                                for j in range(HPC):
                                    ks = slice(HD * j, HD * (j + 1))
                                    ps = psum.tile([BT, 512], F32, name="psS",
                                                   tag="psS",
                                                   bufs=2 if DMATR else 1)
                                    nc.tensor.matmul(
                                        ps[:, 0:w], kTt[ks, kcols],
                                        qTt[ks, qcols], start=True, stop=True,
                                        tile_position=(HD * j, 0))
                                    nc.scalar.activation(
                                        pT[j][:, c0:c1], ps[:, 0:w],
                                        EXP, scale=0.125)
                        # partial-block masking (0/1 pattern multiply on DVE)
                        for (pkb, qc, pidx) in parts:
                            if pkb != kb:
                                continue
                            for j in range(HPC):
                                slc = pT[j][:, qc:qc + BT]
                                nc.vector.tensor_mul(slc, slc, pmt[pidx][:])
                        # AV accumulate (+ denominator via ones column)
                        for j in range(HPC):
                            vt = v1h[(j, b * NB + kb)]
                            for (c0, c1) in pieces:
                                cc = c0 // 512
                                nc.tensor.matmul(
                                    psO[j][:, c0:c1], vt[:],
                                    pT[j][:, c0:c1],
                                    start=(kb == fl[cc][0]),
                                    stop=(kb == fl[cc][1]))
                        if kb == fl[0][1] and fl[0][1] != fl[1][1]:
                            emit_half(0)
                    if fl[0][1] == fl[1][1]:
                        emit_half(0)
                    emit_half(1)
                    if s >= 1:
                        emit_outproj(s - 1)
                emit_outproj(NSEC - 1)
    nc.compile()
    return nc


_CACHE = {}


def _get_nc(plan_key, mask):
    if plan_key not in _CACHE:
        _CACHE[plan_key] = build_nc(make_plan(mask))
    return _CACHE[plan_key]


def _prep_inputs(x, mask, Wq, Wk, Wv, Wo, bo, plan):
    xT = np.ascontiguousarray(x.reshape(TOK, D).T).astype(BF)
    woT = np.ascontiguousarray(Wo.T).astype(BF)
    bo1 = np.ascontiguousarray(bo[None, :]).astype(np.float32)
    in_maps = []
    for c in range(NCORES):
        rows = slice(BT * c, BT * (c + 1))
        in_maps.append({
            "xT": xT,
            "wqT": np.ascontiguousarray(Wq[rows].T).astype(BF),
            "wkT": np.ascontiguousarray(Wk[rows].T).astype(BF),
            "wvT": np.ascontiguousarray(Wv[rows].T).astype(BF),
            "woT": woT,
            "bo1": bo1,
            "pm": plan["patterns"],
            "ident": np.eye(BT, dtype=np.float32),
        })
    return in_maps


def run(inputs, trace=False, **kw):
    x = np.asarray(inputs["x"], np.float32)
    mask = np.asarray(inputs["mask"])
    plan_key = mask.tobytes()
    nc = _get_nc(plan_key, mask)
    plan = make_plan(mask)
    in_maps = _prep_inputs(x, mask, np.asarray(inputs["Wq"], np.float32),
                           np.asarray(inputs["Wk"], np.float32),
                           np.asarray(inputs["Wv"], np.float32),
                           np.asarray(inputs["Wo"], np.float32),
                           np.asarray(inputs["bo"], np.float32), plan)
    res = run_bass_kernel_spmd(nc, in_maps, core_ids=list(range(NCORES)),
                               trace=trace, **kw)
    # unshard: core c, section s, half h, row i ->
    #   global token s*1024 + 512h + 64c + i
    out = np.empty((TOK, D), np.float32)
    for c in range(NCORES):
        yc = res.results[c]["y"]
        for s in range(NSEC):
            for h in range(2):
                out[s * SW + 512 * h + HD * c:
                    s * SW + 512 * h + HD * (c + 1)] = \
                    yc[BT * s + HD * h: BT * s + HD * (h + 1)]
    return out.reshape(B, N, D), res


def kernel(**inputs):
    out, _ = run(inputs, trace=False)
    return out
